# revision 30
# baseline (speedup 1.0000x reference)
"""BEiT attention block (dense_transformer) as a Trainium2 Bass/Tile kernel.

Sharding: head-parallel across 8 NeuronCores. Core c owns heads {2c, 2c+1}
(= qkv channels c*128 .. c*128+127). Each core computes its heads' QKV,
attention, and a partial projection
out_partial = O_heads @ proj_weight[:, c*128:(c+1)*128].T, returned
transposed as [1024, 4100] bf16. Host sums the 8 partials + proj bias
(with v_bias pre-folded into proj_bias on the host, O being linear in v).

Design notes:
  - QT/KT computed in [channel, seq] layout (weights stationary, xT moving)
  - attention scores computed transposed: S[k, q] = K @ Q^T per (batch, head)
  - rel-pos bias applied ADDITIVELY inside the S matmul: rank-64 SVD factors
    (host) ride in the 64 otherwise-unused contraction rows of the per-(b,h)
    K/Q tiles, so S = K.Q + sum_r a_r[k] b_r[q] for free; exp(S) is a single
    scalar-engine activation per tile (no expb multiply, no expb DMA).
  - padded keys killed via V_ext = 0 rows + valid-keys-only ones column
    (softmax sums ride the PV matmul as that ones column).
  - software pipelining: PV(kt-1) emitted after S(kt) so the exp latency
    hides under PE work.
  - normalization: sums row -> reciprocal_approx_fast (vector, needs a
    partition-0 input) -> gpsimd partition_broadcast -> tensor_mul; no DRAM
    round trip.
"""

import os
import sys
import numpy as np

for _p in ("/opt/trn_rl_repo", "/root/.axon_site/_ro/trn_rl_repo"):
    if os.path.isdir(_p) and _p not in sys.path:
        sys.path.insert(0, _p)

import ml_dtypes
from contextlib import ExitStack

import concourse.bass as bass
import concourse.mybir as mybir
import concourse.tile as tile
from concourse import bacc
from concourse.bass_utils import run_bass_kernel_spmd

BF16NP = ml_dtypes.bfloat16
F32 = mybir.dt.float32
BF = mybir.dt.bfloat16

# Problem constants (hardcoded per spec)
B, N, C = 4, 1025, 1024
NH, HD = 16, 64
NCORES = 8
HPC = 2                      # heads per core
BN = B * N                   # 4100
SEQP = 1152                  # per-batch padded seq length (9*128)
KT = 9                       # key tiles (of 128) per batch
NQM = 1024                   # "main" query columns; col 1024 is the tail
PATCH = 16
OLD_WS = (24, 24)
NEW_WS = (32, 32)
VBLK = 80                    # V_ext block stride (64 V cols + 1 ones + pad)
RB = 64                      # rank of the additive rel-pos bias factorization

_CACHE = {}


# ----------------------------------------------------------------------------
# host-side: relative position bias (matches reference bit-for-bit-ish)
# ----------------------------------------------------------------------------

def _gen_relative_position_index(window_size):
    wh, ww = window_size
    num_rel = (2 * wh - 1) * (2 * ww - 1) + 3
    coords = np.stack(np.meshgrid(np.arange(wh), np.arange(ww), indexing='ij'))
    cf = coords.reshape(2, -1)
    rel = cf[:, :, None] - cf[:, None, :]
    rel = rel.transpose(1, 2, 0).astype(np.int64)
    rel[:, :, 0] += wh - 1
    rel[:, :, 1] += ww - 1
    rel[:, :, 0] *= 2 * ww - 1
    n = wh * ww + 1
    rpi = np.zeros((n, n), dtype=np.int64)
    rpi[1:, 1:] = rel.sum(-1)
    rpi[0, 0:] = num_rel - 3
    rpi[0:, 0] = num_rel - 2
    rpi[0, 0] = num_rel - 1
    return rpi


def _rel_pos_bias(table):
    """table [2212, 16] fp32 -> bias [nH, N, N] fp32 (same math as reference)."""
    import jax
    import jax.numpy as jnp

    oh, ow = 2 * OLD_WS[0] - 1, 2 * OLD_WS[1] - 1
    nh_, nw = 2 * NEW_WS[0] - 1, 2 * NEW_WS[1] - 1
    old_num = oh * ow + 3
    new_num = nh_ * nw + 3
    with jax.default_device(jax.devices("cpu")[0]):
        t = jnp.asarray(table)
        sub = t[: old_num - 3].reshape(ow, oh, NH).transpose(2, 0, 1)
        sub = jax.image.resize(sub, (NH, nh_, nw), method='bilinear')
        sub = sub.transpose(1, 2, 0).reshape(new_num - 3, NH)
        new_table = np.asarray(jnp.concatenate([sub, t[old_num - 3:]], axis=0))
    idx = _gen_relative_position_index(NEW_WS)
    bias = new_table[idx.reshape(-1)].reshape(N, N, NH)  # [q, k, h]
    return bias.transpose(2, 0, 1)  # [h, q, k]


def _bias_factors(table):
    """Rank-RB factors: bias[h, q, k] ~= sum_r bfac[h, r, q] * afac[h, r, k]."""
    bias = _rel_pos_bias(table)
    afac = np.zeros((NH, RB, N), dtype=np.float32)
    bfac = np.zeros((NH, RB, N), dtype=np.float32)
    for h in range(NH):
        U, S, Vt = np.linalg.svd(bias[h], full_matrices=False)
        rs = np.sqrt(S[:RB])
        bfac[h] = (U[:, :RB] * rs).T
        afac[h] = (Vt[:RB].T * rs).T
    return afac, bfac


# ----------------------------------------------------------------------------
# device kernel
# ----------------------------------------------------------------------------

def build_nc(repeat=1):
    nc = bacc.Bacc("TRN2", target_bir_lowering=False, debug=False)

    xT = nc.dram_tensor("xT", [C, BN], BF, kind="ExternalInput").ap()
    wqT = nc.dram_tensor("wqT", [128, 8 * 128], BF, kind="ExternalInput").ap()
    wkT = nc.dram_tensor("wkT", [128, 8 * 128], BF, kind="ExternalInput").ap()
    wvT = nc.dram_tensor("wvT", [128, 8 * 128], BF, kind="ExternalInput").ap()
    qb = nc.dram_tensor("qb", [128, 1], F32, kind="ExternalInput").ap()
    kb = nc.dram_tensor("kb", [128, 1], F32, kind="ExternalInput").ap()
    pwT = nc.dram_tensor("pwT", [128, C], BF, kind="ExternalInput").ap()
    afac = nc.dram_tensor("afac", [RB, HPC * SEQP], BF, kind="ExternalInput").ap()
    bfac = nc.dram_tensor("bfac", [RB, HPC * SEQP], BF, kind="ExternalInput").ap()
    outt = nc.dram_tensor("out_t", [C, BN], BF, kind="ExternalOutput").ap()

    with TileCtx(nc) as (tc, ctx):
        singles = ctx.enter_context(tc.tile_pool(name="singles", bufs=1))

        qex = [singles.tile([128, SEQP], BF, name=f"qex{u}") for u in range(B * HPC)]
        kex = [singles.tile([128, SEQP], BF, name=f"kex{u}") for u in range(B * HPC)]
        ve_sb = [singles.tile([128, KT * HPC * VBLK], BF, name=f"ve_sb{b}")
                 for b in range(B)]
        otall = [singles.tile([128, N], BF, name=f"otall{b}") for b in range(B)]
        pw_sb = singles.tile([128, C], BF, name="pw_sb")
        qb_sb = singles.tile([128, 1], F32, name="qb_sb")
        kb_sb = singles.tile([128, 1], F32, name="kb_sb")
        ident_sb = singles.tile([128, 128], BF, name="ident_sb")

        # packed weight tiles first (QKV critical path; [p, kc, cout], one
        # DMA each), then x batch-by-batch.
        wq_sb = singles.tile([128, 8 * 128], BF, name="wq_sb")
        wk_sb = singles.tile([128, 8 * 128], BF, name="wk_sb")
        wv_sb = singles.tile([128, 8 * 128], BF, name="wv_sb")
        nc.sync.dma_start(out=wq_sb, in_=wqT)
        nc.sync.dma_start(out=wk_sb, in_=wkT)
        nc.sync.dma_start(out=wv_sb, in_=wvT)
        nc.sync.dma_start(out=qb_sb, in_=qb)
        nc.sync.dma_start(out=kb_sb, in_=kb)
        wq_t = [wq_sb[:, kc * 128:(kc + 1) * 128] for kc in range(8)]
        wk_t = [wk_sb[:, kc * 128:(kc + 1) * 128] for kc in range(8)]
        wv_t = [wv_sb[:, kc * 128:(kc + 1) * 128] for kc in range(8)]

        xts = [[singles.tile([128, N], BF, name=f"x{b}_{kc}") for kc in range(8)]
               for b in range(B)]
        for b in range(B):
            for kc in range(8):
                nc.sync.dma_start(
                    out=xts[b][kc],
                    in_=xT[kc * 128:(kc + 1) * 128, b * N:(b + 1) * N])

        # bias factor rows + pw on the gpsimd queue (sync is the busy one)
        for u in range(B * HPC):
            h = u % HPC
            nc.gpsimd.dma_start(out=kex[u][64:64 + RB, :],
                                in_=afac[:, h * SEQP:(h + 1) * SEQP])
            nc.gpsimd.dma_start(out=qex[u][64:64 + RB, :],
                                in_=bfac[:, h * SEQP:(h + 1) * SEQP])
            nc.gpsimd.memset(kex[u][0:64, N:SEQP], 0.0)
        nc.gpsimd.dma_start(out=pw_sb, in_=pwT)

        # V_ext: zero (pad-key kill), ones only on valid-key rows of the
        # sums column of each (kt, h) block.
        for b in range(B):
            nc.vector.memset(ve_sb[b], 0.0)
        for b in range(B):
            for kt in range(KT):
                stw = 128 if kt < 8 else 1
                for h in range(HPC):
                    col = (kt * HPC + h) * VBLK + 64
                    nc.gpsimd.memset(ve_sb[b][:stw, col:col + 1], 1.0)
        from concourse.masks import make_identity
        make_identity(nc, ident_sb)

        for _rep in range(repeat):
            _emit_phases(nc, tc, qex, kex, ve_sb, otall, pw_sb,
                         qb_sb, kb_sb, ident_sb, wq_t, wk_t, wv_t, xts, outt)

    nc.compile()
    return nc


def _emit_phases(nc, tc, qex, kex, ve_sb, otall, pw_sb,
                 qb_sb, kb_sb, ident_sb, wq_t, wk_t, wv_t, xts, outt):
    EXP = mybir.ActivationFunctionType.Exp

    # ------------------------- QKV phase (batch-local) -----------------------
    with tc.tile_pool(name="vtmp", bufs=2) as vtpool, \
         tc.tile_pool(name="qkv_ps", bufs=3, space="PSUM") as qkps, \
         tc.tile_pool(name="tp_ps", bufs=2, space="PSUM") as tppool:
        for b in range(B):
            vt_b = vtpool.tile([128, N], BF, tag="vt")
            for (w_t, kind, bias_col) in (
                    (wq_t, "q", qb_sb), (wk_t, "k", kb_sb), (wv_t, "v", None)):
                for (c0, cw) in ((0, 512), (512, 512), (1024, 1)):
                    ps = qkps.tile([128, 512], F32, tag="qkv")
                    for kc in range(8):
                        nc.tensor.matmul(ps[:, :cw], w_t[kc],
                                         xts[b][kc][:, c0:c0 + cw],
                                         start=(kc == 0), stop=(kc == 7))
                    if kind == "v":
                        nc.vector.tensor_copy(vt_b[:, c0:c0 + cw], ps[:, :cw])
                    else:
                        dsts = qex if kind == "q" else kex
                        nc.vector.tensor_scalar_add(
                            dsts[2 * b][0:64, c0:c0 + cw], ps[0:64, :cw],
                            bias_col[0:64, :])
                        nc.scalar.add(
                            dsts[2 * b + 1][0:64, c0:c0 + cw], ps[64:128, :cw],
                            bias_col[64:128, :])
            for kt in range(KT):
                stw = 128 if kt < 8 else 1
                vp = tppool.tile([128, 128], BF, tag="tp")
                nc.tensor.transpose(vp[:stw, :],
                                    vt_b[:, kt * 128: kt * 128 + stw], ident_sb)
                # both heads' V in one copy: dst cols {0..63} u {VBLK..VBLK+63}
                vdst = ve_sb[b][:stw, kt * HPC * VBLK: kt * HPC * VBLK + VBLK + 64]
                vdst = bass.AP(tensor=vdst.tensor, offset=vdst.offset,
                               ap=list(vdst.ap[:-1]) + [[VBLK, 2], [1, 64]])
                vsrc = vp[:stw, :]
                vsrc = bass.AP(tensor=vsrc.tensor, offset=vsrc.offset,
                               ap=list(vsrc.ap[:-1]) + [[64, 2], [1, 64]])
                nc.vector.tensor_copy(vdst, vsrc)

    # ------------------------- attention phase -------------------------------
    with tc.tile_pool(name="s_ps", bufs=2, space="PSUM") as sps, \
         tc.tile_pool(name="ot_ps", bufs=1, space="PSUM") as otps, \
         tc.tile_pool(name="tail_ps", bufs=2, space="PSUM") as tailps, \
         tc.tile_pool(name="pp", bufs=3) as ppool, \
         tc.tile_pool(name="ptail", bufs=2) as ptpool, \
         tc.tile_pool(name="otraw", bufs=3) as orpool, \
         tc.tile_pool(name="sums", bufs=2) as smpool, \
         tc.tile_pool(name="rbc", bufs=2) as rbcpool:

        for u in range(B * HPC):
            b, h = u // HPC, u % HPC
            hp = h * 64

            def k_lhs(kt):
                return kex[u][:, kt * 128:(kt + 1) * 128]

            def ve_lhs(kt):
                blk = (kt * HPC + h) * VBLK
                return ve_sb[b][:, blk: blk + 65]

            # tail query column (q = 1024)
            ot_c = tailps.tile([65, 1], F32, tag="otc")
            s_tail = sps.tile([128, NQM], F32, tag="s")
            for kt in range(KT):
                nc.tensor.matmul(s_tail[:, kt:kt + 1], k_lhs(kt),
                                 qex[u][:, NQM:NQM + 1], start=True, stop=True)
            ptm = ptpool.tile([128, KT], BF, tag="ptm")
            nc.scalar.activation(ptm, s_tail[:, 0:KT], EXP)

            # main loop, software-pipelined: PV(kt-1) after S(kt)
            ot = otps.tile([65, NQM], F32, tag="ot")
            ps_ = [None] * KT

            def emit_s(kt):
                s = sps.tile([128, NQM], F32, tag="s")
                nc.tensor.matmul(s[:, 0:512], k_lhs(kt), qex[u][:, 0:512],
                                 start=True, stop=True)
                nc.tensor.matmul(s[:, 512:1024], k_lhs(kt), qex[u][:, 512:1024],
                                 start=True, stop=True)
                p = ppool.tile([128, NQM], BF, tag="p")
                nc.scalar.activation(p, s, EXP)
                ps_[kt] = p

            def emit_pv(kt):
                p = ps_[kt]
                nc.tensor.matmul(ot[:, 0:512], ve_lhs(kt), p[:, 0:512],
                                 start=(kt == 0), stop=(kt == KT - 1))
                nc.tensor.matmul(ot[:, 512:1024], ve_lhs(kt), p[:, 512:1024],
                                 start=(kt == 0), stop=(kt == KT - 1))
                nc.tensor.matmul(ot_c, ve_lhs(kt), ptm[:, kt:kt + 1],
                                 start=(kt == 0), stop=(kt == KT - 1))

            emit_s(0)
            for kt in range(1, KT):
                emit_s(kt)
                emit_pv(kt - 1)
            emit_pv(KT - 1)

            # epilogue: evacuate psum fast, then normalize; sums land on
            # partition 0 (reciprocal_approx_fast needs a partition-0 input).
            otraw = orpool.tile([64, N], F32, tag="otraw")
            sums = smpool.tile([1, N], F32, tag="sums")
            nc.vector.tensor_copy(sums[:, 0:1024], ot[64:65, :])
            nc.vector.tensor_copy(sums[:, 1024:1025], ot_c[64:65, :])
            nc.vector.tensor_copy(otraw[:, 0:1024], ot[0:64, :])
            nc.vector.tensor_copy(otraw[:, 1024:1025], ot_c[0:64, :])
            rr = smpool.tile([1, N], F32, tag="rr")
            nc.vector.reciprocal_approx_fast(rr, sums)
            rbc = rbcpool.tile([64, N], F32, tag="rbc")
            nc.gpsimd.partition_broadcast(rbc, rr)
            nc.vector.tensor_mul(otall[b][hp:hp + 64, :], otraw, rbc)

    # ------------------------- projection phase ------------------------------
    with tc.tile_pool(name="pj_ps", bufs=6, space="PSUM") as pjps, \
         tc.tile_pool(name="osb", bufs=4) as opool:
        for b in range(B):
            for ct in range(8):
                osb = opool.tile([128, N], BF, tag="osb")
                for ci, (q0, qw) in enumerate(((0, 512), (512, 512), (1024, 1))):
                    pj = pjps.tile([128, 512], F32, tag="pj")
                    nc.tensor.matmul(pj[:, :qw], pw_sb[:, ct * 128:(ct + 1) * 128],
                                     otall[b][:, q0:q0 + qw], start=True, stop=True)
                    if ci == 1:
                        nc.scalar.copy(osb[:, q0:q0 + qw], pj[:, :qw])
                    else:
                        nc.vector.tensor_copy(osb[:, q0:q0 + qw], pj[:, :qw])
                nc.gpsimd.dma_start(
                    out=outt[ct * 128:(ct + 1) * 128, b * N:(b + 1) * N],
                    in_=osb)


class TileCtx:
    """with TileCtx(nc) as (tc, ctx): ... (TileContext + ExitStack combined)."""

    def __init__(self, nc):
        self.nc = nc

    def __enter__(self):
        self._tc = tile.TileContext(self.nc)
        self._ctx = ExitStack()
        tc = self._tc.__enter__()
        ctx = self._ctx.__enter__()
        return tc, ctx

    def __exit__(self, *exc):
        self._ctx.__exit__(*exc)
        return self._tc.__exit__(*exc)


# ----------------------------------------------------------------------------
# host-side input prep / output gather
# ----------------------------------------------------------------------------

def _prep_inputs(x, qkv_weight, q_bias, k_bias, v_bias, proj_weight, rel_pos_table):
    """Returns in_maps (list of 8 dicts)."""
    scale = (C // NH) ** -0.5  # 0.125

    xT = np.ascontiguousarray(
        np.asarray(x, dtype=np.float32).reshape(BN, C).T).astype(BF16NP)

    tbl = np.asarray(rel_pos_table, dtype=np.float32)
    key = tbl.tobytes()[:64]
    if _CACHE.get("fac_key") != key:
        _CACHE["afac"], _CACHE["bfac"] = _bias_factors(tbl)
        _CACHE["fac_key"] = key
    afac_all, bfac_all = _CACHE["afac"], _CACHE["bfac"]

    qkv_w = np.asarray(qkv_weight, dtype=np.float32)
    qb_full = np.asarray(q_bias, dtype=np.float32)
    kb_full = np.asarray(k_bias, dtype=np.float32)
    pw = np.asarray(proj_weight, dtype=np.float32)

    in_maps = []
    for c in range(NCORES):
        sl = slice(c * 128, (c + 1) * 128)
        def pk(w):  # [128 out, 1024 in] -> [p, kc, cout]
            return np.ascontiguousarray(
                w.T.reshape(8, 128, 128).transpose(1, 0, 2).reshape(128, 8 * 128))
        wq = pk(qkv_w[0 * C:1 * C][sl] * scale)
        wk = pk(qkv_w[1 * C:2 * C][sl])
        wv = pk(qkv_w[2 * C:3 * C][sl])
        af = np.zeros((RB, HPC * SEQP), dtype=np.float32)
        bf = np.zeros((RB, HPC * SEQP), dtype=np.float32)
        for h in range(HPC):
            af[:, h * SEQP: h * SEQP + N] = afac_all[2 * c + h]
            bf[:, h * SEQP: h * SEQP + N] = bfac_all[2 * c + h]
        in_maps.append({
            "xT": xT,
            "wqT": np.ascontiguousarray(wq).astype(BF16NP),
            "wkT": np.ascontiguousarray(wk).astype(BF16NP),
            "wvT": np.ascontiguousarray(wv).astype(BF16NP),
            "qb": np.ascontiguousarray((qb_full[sl] * scale).reshape(128, 1)),
            "kb": np.ascontiguousarray(kb_full[sl].reshape(128, 1)),
            "pwT": np.ascontiguousarray(pw[:, sl].T).astype(BF16NP),
            "afac": np.ascontiguousarray(af).astype(BF16NP),
            "bfac": np.ascontiguousarray(bf).astype(BF16NP),
        })
    return in_maps


LAST_RESULTS = None


def kernel(x, qkv_weight, q_bias, k_bias, v_bias, proj_weight, proj_bias,
           rel_pos_table, res_h=512, res_w=512):
    global LAST_RESULTS
    if "nc" not in _CACHE:
        _CACHE["nc"] = build_nc()
    nc = _CACHE["nc"]

    in_maps = _prep_inputs(x, qkv_weight, q_bias, k_bias, v_bias, proj_weight,
                           rel_pos_table)
    trace = os.environ.get("KERNEL_TRACE", "0") == "1"
    res = run_bass_kernel_spmd(nc, in_maps, core_ids=list(range(NCORES)),
                               trace=trace)
    LAST_RESULTS = res

    total = np.zeros((C, BN), dtype=np.float32)
    for r in res.results:
        total += np.asarray(r["out_t"], dtype=np.float32)
    # v_bias is linear through attention + projection: fold on host.
    bias_eff = (np.asarray(proj_bias, dtype=np.float32)
                + np.asarray(proj_weight, dtype=np.float32)
                @ np.asarray(v_bias, dtype=np.float32))
    out = total.T + bias_eff
    return np.ascontiguousarray(out.reshape(B, N, C), dtype=np.float32)


# revision 31
# speedup vs baseline: 1.1751x; 1.1751x over previous
"""BEiT attention block (dense_transformer) as a Trainium2 Bass/Tile kernel.

Sharding: head-parallel across 8 NeuronCores. Core c owns heads {2c, 2c+1}
(= qkv channels c*128 .. c*128+127). Each core computes its heads' QKV,
attention, and a partial projection
out_partial = O_heads @ proj_weight[:, c*128:(c+1)*128].T, returned
transposed as [1024, 4100] bf16. Host sums the 8 partials + proj bias
(with v_bias pre-folded into proj_bias on the host, O being linear in v).

Design notes:
  - QT/KT computed in [channel, seq] layout (weights stationary, xT moving)
  - attention scores computed transposed: S[k, q] = K @ Q^T per (batch, head)
  - rel-pos bias applied ADDITIVELY inside the S matmul: rank-64 SVD factors
    (host) ride in the 64 otherwise-unused contraction rows of the per-(b,h)
    K/Q tiles, so S = K.Q + sum_r a_r[k] b_r[q] for free; exp(S) is a single
    scalar-engine activation per tile (no expb multiply, no expb DMA).
  - padded keys killed via V_ext = 0 rows + valid-keys-only ones column
    (softmax sums ride the PV matmul as that ones column).
  - software pipelining: PV(kt-1) emitted after S(kt) so the exp latency
    hides under PE work.
  - normalization: sums row -> reciprocal_approx_fast (vector, needs a
    partition-0 input) -> gpsimd partition_broadcast -> tensor_mul; no DRAM
    round trip.
"""

import os
import sys
import numpy as np

for _p in ("/opt/trn_rl_repo", "/root/.axon_site/_ro/trn_rl_repo"):
    if os.path.isdir(_p) and _p not in sys.path:
        sys.path.insert(0, _p)

import ml_dtypes
from contextlib import ExitStack

import concourse.bass as bass
import concourse.mybir as mybir
import concourse.tile as tile
from concourse import bacc
from concourse.bass_utils import run_bass_kernel_spmd

BF16NP = ml_dtypes.bfloat16
F32 = mybir.dt.float32
BF = mybir.dt.bfloat16

# Problem constants (hardcoded per spec)
B, N, C = 4, 1025, 1024
NH, HD = 16, 64
NCORES = 8
HPC = 2                      # heads per core
BN = B * N                   # 4100
SEQP = 1152                  # per-batch padded seq length (9*128)
KT = 9                       # key tiles (of 128) per batch
NQM = 1024                   # "main" query columns; col 1024 is the tail
PATCH = 16
OLD_WS = (24, 24)
NEW_WS = (32, 32)
VBLK = 80                    # V_ext block stride (64 V cols + 1 ones + pad)
RB = 64                      # rank of the additive rel-pos bias factorization

_CACHE = {}


# ----------------------------------------------------------------------------
# host-side: relative position bias (matches reference bit-for-bit-ish)
# ----------------------------------------------------------------------------

def _gen_relative_position_index(window_size):
    wh, ww = window_size
    num_rel = (2 * wh - 1) * (2 * ww - 1) + 3
    coords = np.stack(np.meshgrid(np.arange(wh), np.arange(ww), indexing='ij'))
    cf = coords.reshape(2, -1)
    rel = cf[:, :, None] - cf[:, None, :]
    rel = rel.transpose(1, 2, 0).astype(np.int64)
    rel[:, :, 0] += wh - 1
    rel[:, :, 1] += ww - 1
    rel[:, :, 0] *= 2 * ww - 1
    n = wh * ww + 1
    rpi = np.zeros((n, n), dtype=np.int64)
    rpi[1:, 1:] = rel.sum(-1)
    rpi[0, 0:] = num_rel - 3
    rpi[0:, 0] = num_rel - 2
    rpi[0, 0] = num_rel - 1
    return rpi


def _rel_pos_bias(table):
    """table [2212, 16] fp32 -> bias [nH, N, N] fp32 (same math as reference)."""
    import jax
    import jax.numpy as jnp

    oh, ow = 2 * OLD_WS[0] - 1, 2 * OLD_WS[1] - 1
    nh_, nw = 2 * NEW_WS[0] - 1, 2 * NEW_WS[1] - 1
    old_num = oh * ow + 3
    new_num = nh_ * nw + 3
    with jax.default_device(jax.devices("cpu")[0]):
        t = jnp.asarray(table)
        sub = t[: old_num - 3].reshape(ow, oh, NH).transpose(2, 0, 1)
        sub = jax.image.resize(sub, (NH, nh_, nw), method='bilinear')
        sub = sub.transpose(1, 2, 0).reshape(new_num - 3, NH)
        new_table = np.asarray(jnp.concatenate([sub, t[old_num - 3:]], axis=0))
    idx = _gen_relative_position_index(NEW_WS)
    bias = new_table[idx.reshape(-1)].reshape(N, N, NH)  # [q, k, h]
    return bias.transpose(2, 0, 1)  # [h, q, k]


def _bias_factors(table):
    """Rank-RB factors: bias[h, q, k] ~= sum_r bfac[h, r, q] * afac[h, r, k]."""
    bias = _rel_pos_bias(table)
    afac = np.zeros((NH, RB, N), dtype=np.float32)
    bfac = np.zeros((NH, RB, N), dtype=np.float32)
    for h in range(NH):
        U, S, Vt = np.linalg.svd(bias[h], full_matrices=False)
        rs = np.sqrt(S[:RB])
        bfac[h] = (U[:, :RB] * rs).T
        afac[h] = (Vt[:RB].T * rs).T
    return afac, bfac


# ----------------------------------------------------------------------------
# device kernel
# ----------------------------------------------------------------------------

def build_nc(repeat=1):
    nc = bacc.Bacc("TRN2", target_bir_lowering=False, debug=False)

    xT = nc.dram_tensor("xT", [C, BN], BF, kind="ExternalInput").ap()
    wqT = nc.dram_tensor("wqT", [128, 8 * 128], BF, kind="ExternalInput").ap()
    wkT = nc.dram_tensor("wkT", [128, 8 * 128], BF, kind="ExternalInput").ap()
    wvT = nc.dram_tensor("wvT", [128, 8 * 128], BF, kind="ExternalInput").ap()
    qb = nc.dram_tensor("qb", [128, 1], F32, kind="ExternalInput").ap()
    kb = nc.dram_tensor("kb", [128, 1], F32, kind="ExternalInput").ap()
    pwT = nc.dram_tensor("pwT", [128, C], BF, kind="ExternalInput").ap()
    afac = nc.dram_tensor("afac", [RB, HPC * SEQP], BF, kind="ExternalInput").ap()
    bfac = nc.dram_tensor("bfac", [RB, HPC * SEQP], BF, kind="ExternalInput").ap()
    outt = nc.dram_tensor("out_t", [C, BN], BF, kind="ExternalOutput").ap()

    with TileCtx(nc) as (tc, ctx):
        singles = ctx.enter_context(tc.tile_pool(name="singles", bufs=1))

        qex = [singles.tile([128, SEQP], BF, name=f"qex{u}") for u in range(B * HPC)]
        kex = [singles.tile([128, SEQP], BF, name=f"kex{u}") for u in range(B * HPC)]
        ve_sb = [singles.tile([128, KT * HPC * VBLK], BF, name=f"ve_sb{b}")
                 for b in range(B)]
        otall = [singles.tile([128, N], BF, name=f"otall{b}") for b in range(B)]
        pw_sb = singles.tile([128, C], BF, name="pw_sb")
        qb_sb = singles.tile([128, 1], F32, name="qb_sb")
        kb_sb = singles.tile([128, 1], F32, name="kb_sb")
        ident_sb = singles.tile([128, 128], BF, name="ident_sb")

        # packed weight tiles first (QKV critical path; [p, kc, cout], one
        # DMA each), then x batch-by-batch.
        wq_sb = singles.tile([128, 8 * 128], BF, name="wq_sb")
        wk_sb = singles.tile([128, 8 * 128], BF, name="wk_sb")
        wv_sb = singles.tile([128, 8 * 128], BF, name="wv_sb")
        nc.sync.dma_start(out=wq_sb, in_=wqT)
        nc.sync.dma_start(out=wk_sb, in_=wkT)
        nc.sync.dma_start(out=wv_sb, in_=wvT)
        nc.sync.dma_start(out=qb_sb, in_=qb)
        nc.sync.dma_start(out=kb_sb, in_=kb)
        wq_t = [wq_sb[:, kc * 128:(kc + 1) * 128] for kc in range(8)]
        wk_t = [wk_sb[:, kc * 128:(kc + 1) * 128] for kc in range(8)]
        wv_t = [wv_sb[:, kc * 128:(kc + 1) * 128] for kc in range(8)]

        xts = [[singles.tile([128, N], BF, name=f"x{b}_{kc}") for kc in range(8)]
               for b in range(B)]
        for b in range(B):
            for kc in range(8):
                src_rows = xT[kc * 128:(kc + 1) * 128, :]
                if b == 0:
                    # halve batch-0 transfers: 2x DMA-engine parallelism on
                    # the first-matmul critical path
                    nc.sync.dma_start(out=xts[b][kc][:, 0:512],
                                      in_=src_rows[:, b * N: b * N + 512])
                    nc.sync.dma_start(out=xts[b][kc][:, 512:N],
                                      in_=src_rows[:, b * N + 512:(b + 1) * N])
                else:
                    nc.sync.dma_start(out=xts[b][kc],
                                      in_=src_rows[:, b * N:(b + 1) * N])

        # bias factor rows + pw on the gpsimd queue (sync is the busy one)
        for u in range(B * HPC):
            h = u % HPC
            nc.gpsimd.dma_start(out=kex[u][64:64 + RB, :],
                                in_=afac[:, h * SEQP:(h + 1) * SEQP])
            nc.gpsimd.dma_start(out=qex[u][64:64 + RB, :],
                                in_=bfac[:, h * SEQP:(h + 1) * SEQP])
            nc.gpsimd.memset(kex[u][0:64, N:SEQP], 0.0)
        nc.gpsimd.dma_start(out=pw_sb, in_=pwT)

        # V_ext: zero (pad-key kill), ones only on valid-key rows of the
        # sums column of each (kt, h) block.
        for b in range(B):
            nc.vector.memset(ve_sb[b], 0.0)
        for b in range(B):
            for kt in range(KT):
                stw = 128 if kt < 8 else 1
                for h in range(HPC):
                    col = (kt * HPC + h) * VBLK + 64
                    nc.gpsimd.memset(ve_sb[b][:stw, col:col + 1], 1.0)
        from concourse.masks import make_identity
        make_identity(nc, ident_sb)

        for _rep in range(repeat):
            _emit_phases(nc, tc, qex, kex, ve_sb, otall, pw_sb,
                         qb_sb, kb_sb, ident_sb, wq_t, wk_t, wv_t, xts, outt)

    nc.compile()
    return nc


def _emit_phases(nc, tc, qex, kex, ve_sb, otall, pw_sb,
                 qb_sb, kb_sb, ident_sb, wq_t, wk_t, wv_t, xts, outt):
    EXP = mybir.ActivationFunctionType.Exp

    # ------------------------- QKV phase (batch-local) -----------------------
    with tc.tile_pool(name="vtmp", bufs=2) as vtpool, \
         tc.tile_pool(name="qkv_ps", bufs=3, space="PSUM") as qkps, \
         tc.tile_pool(name="tp_ps", bufs=2, space="PSUM") as tppool:
        for b in range(B):
            vt_b = vtpool.tile([128, N], BF, tag="vt")
            for (w_t, kind, bias_col) in (
                    (wq_t, "q", qb_sb), (wk_t, "k", kb_sb), (wv_t, "v", None)):
                for (c0, cw) in ((0, 512), (512, 512), (1024, 1)):
                    ps = qkps.tile([128, 512], F32, tag="qkv")
                    for kc in range(8):
                        nc.tensor.matmul(ps[:, :cw], w_t[kc],
                                         xts[b][kc][:, c0:c0 + cw],
                                         start=(kc == 0), stop=(kc == 7))
                    if kind == "v":
                        nc.vector.tensor_copy(vt_b[:, c0:c0 + cw], ps[:, :cw])
                    else:
                        dsts = qex if kind == "q" else kex
                        nc.vector.tensor_scalar_add(
                            dsts[2 * b][0:64, c0:c0 + cw], ps[0:64, :cw],
                            bias_col[0:64, :])
                        nc.scalar.add(
                            dsts[2 * b + 1][0:64, c0:c0 + cw], ps[64:128, :cw],
                            bias_col[64:128, :])
            for kt in range(KT):
                stw = 128 if kt < 8 else 1
                vp = tppool.tile([128, 128], BF, tag="tp")
                nc.tensor.transpose(vp[:stw, :],
                                    vt_b[:, kt * 128: kt * 128 + stw], ident_sb)
                # both heads' V in one copy: dst cols {0..63} u {VBLK..VBLK+63}
                vdst = ve_sb[b][:stw, kt * HPC * VBLK: kt * HPC * VBLK + VBLK + 64]
                vdst = bass.AP(tensor=vdst.tensor, offset=vdst.offset,
                               ap=list(vdst.ap[:-1]) + [[VBLK, 2], [1, 64]])
                vsrc = vp[:stw, :]
                vsrc = bass.AP(tensor=vsrc.tensor, offset=vsrc.offset,
                               ap=list(vsrc.ap[:-1]) + [[64, 2], [1, 64]])
                nc.vector.tensor_copy(vdst, vsrc)

    # ------------------------- attention phase -------------------------------
    with tc.tile_pool(name="s_ps", bufs=2, space="PSUM") as sps, \
         tc.tile_pool(name="ot_ps", bufs=1, space="PSUM") as otps, \
         tc.tile_pool(name="tail_ps", bufs=2, space="PSUM") as tailps, \
         tc.tile_pool(name="pp", bufs=3) as ppool, \
         tc.tile_pool(name="ptail", bufs=2) as ptpool, \
         tc.tile_pool(name="otraw", bufs=3) as orpool, \
         tc.tile_pool(name="sums", bufs=2) as smpool, \
         tc.tile_pool(name="rbc", bufs=2) as rbcpool:

        for u in range(B * HPC):
            b, h = u // HPC, u % HPC
            hp = h * 64

            def k_lhs(kt):
                return kex[u][:, kt * 128:(kt + 1) * 128]

            def ve_lhs(kt):
                blk = (kt * HPC + h) * VBLK
                return ve_sb[b][:, blk: blk + 65]

            # tail query column (q = 1024)
            ot_c = tailps.tile([65, 1], F32, tag="otc")
            s_tail = sps.tile([128, NQM], F32, tag="s")
            for kt in range(KT):
                nc.tensor.matmul(s_tail[:, kt:kt + 1], k_lhs(kt),
                                 qex[u][:, NQM:NQM + 1], start=True, stop=True)
            ptm = ptpool.tile([128, KT], BF, tag="ptm")
            nc.scalar.activation(ptm, s_tail[:, 0:KT], EXP)

            # main loop, software-pipelined: PV(kt-1) after S(kt)
            ot = otps.tile([65, NQM], F32, tag="ot")
            ps_ = [None] * KT

            def emit_s(kt):
                s = sps.tile([128, NQM], F32, tag="s")
                nc.tensor.matmul(s[:, 0:512], k_lhs(kt), qex[u][:, 0:512],
                                 start=True, stop=True)
                nc.tensor.matmul(s[:, 512:1024], k_lhs(kt), qex[u][:, 512:1024],
                                 start=True, stop=True)
                p = ppool.tile([128, NQM], BF, tag="p")
                nc.scalar.activation(p, s, EXP)
                ps_[kt] = p

            def emit_pv(kt):
                p = ps_[kt]
                nc.tensor.matmul(ot[:, 0:512], ve_lhs(kt), p[:, 0:512],
                                 start=(kt == 0), stop=(kt == KT - 1))
                nc.tensor.matmul(ot[:, 512:1024], ve_lhs(kt), p[:, 512:1024],
                                 start=(kt == 0), stop=(kt == KT - 1))
                nc.tensor.matmul(ot_c, ve_lhs(kt), ptm[:, kt:kt + 1],
                                 start=(kt == 0), stop=(kt == KT - 1))

            emit_s(0)
            for kt in range(1, KT):
                emit_s(kt)
                emit_pv(kt - 1)
            emit_pv(KT - 1)

            # epilogue: evacuate psum fast, then normalize; sums land on
            # partition 0 (reciprocal_approx_fast needs a partition-0 input).
            otraw = orpool.tile([64, N], F32, tag="otraw")
            sums = smpool.tile([1, N], F32, tag="sums")
            nc.vector.tensor_copy(sums[:, 0:1024], ot[64:65, :])
            nc.vector.tensor_copy(sums[:, 1024:1025], ot_c[64:65, :])
            nc.vector.tensor_copy(otraw[:, 0:1024], ot[0:64, :])
            nc.vector.tensor_copy(otraw[:, 1024:1025], ot_c[0:64, :])
            rr = smpool.tile([1, N], F32, tag="rr")
            nc.vector.reciprocal_approx_fast(rr, sums)
            rbc = rbcpool.tile([64, N], F32, tag="rbc")
            nc.gpsimd.partition_broadcast(rbc, rr)
            nc.vector.tensor_mul(otall[b][hp:hp + 64, :], otraw, rbc)

    # ------------------------- projection phase ------------------------------
    with tc.tile_pool(name="pj_ps", bufs=6, space="PSUM") as pjps, \
         tc.tile_pool(name="osb", bufs=4) as opool:
        for b in range(B):
            for ct in range(8):
                osb = opool.tile([128, N], BF, tag="osb")
                for ci, (q0, qw) in enumerate(((0, 512), (512, 512), (1024, 1))):
                    pj = pjps.tile([128, 512], F32, tag="pj")
                    nc.tensor.matmul(pj[:, :qw], pw_sb[:, ct * 128:(ct + 1) * 128],
                                     otall[b][:, q0:q0 + qw], start=True, stop=True)
                    if ci == 1:
                        nc.scalar.copy(osb[:, q0:q0 + qw], pj[:, :qw])
                    else:
                        nc.vector.tensor_copy(osb[:, q0:q0 + qw], pj[:, :qw])
                nc.gpsimd.dma_start(
                    out=outt[ct * 128:(ct + 1) * 128, b * N:(b + 1) * N],
                    in_=osb)


class TileCtx:
    """with TileCtx(nc) as (tc, ctx): ... (TileContext + ExitStack combined)."""

    def __init__(self, nc):
        self.nc = nc

    def __enter__(self):
        self._tc = tile.TileContext(self.nc)
        self._ctx = ExitStack()
        tc = self._tc.__enter__()
        ctx = self._ctx.__enter__()
        return tc, ctx

    def __exit__(self, *exc):
        self._ctx.__exit__(*exc)
        return self._tc.__exit__(*exc)


# ----------------------------------------------------------------------------
# host-side input prep / output gather
# ----------------------------------------------------------------------------

def _prep_inputs(x, qkv_weight, q_bias, k_bias, v_bias, proj_weight, rel_pos_table):
    """Returns in_maps (list of 8 dicts)."""
    scale = (C // NH) ** -0.5  # 0.125

    xT = np.ascontiguousarray(
        np.asarray(x, dtype=np.float32).reshape(BN, C).T).astype(BF16NP)

    tbl = np.asarray(rel_pos_table, dtype=np.float32)
    key = tbl.tobytes()[:64]
    if _CACHE.get("fac_key") != key:
        _CACHE["afac"], _CACHE["bfac"] = _bias_factors(tbl)
        _CACHE["fac_key"] = key
    afac_all, bfac_all = _CACHE["afac"], _CACHE["bfac"]

    qkv_w = np.asarray(qkv_weight, dtype=np.float32)
    qb_full = np.asarray(q_bias, dtype=np.float32)
    kb_full = np.asarray(k_bias, dtype=np.float32)
    pw = np.asarray(proj_weight, dtype=np.float32)

    in_maps = []
    for c in range(NCORES):
        sl = slice(c * 128, (c + 1) * 128)
        def pk(w):  # [128 out, 1024 in] -> [p, kc, cout]
            return np.ascontiguousarray(
                w.T.reshape(8, 128, 128).transpose(1, 0, 2).reshape(128, 8 * 128))
        wq = pk(qkv_w[0 * C:1 * C][sl] * scale)
        wk = pk(qkv_w[1 * C:2 * C][sl])
        wv = pk(qkv_w[2 * C:3 * C][sl])
        af = np.zeros((RB, HPC * SEQP), dtype=np.float32)
        bf = np.zeros((RB, HPC * SEQP), dtype=np.float32)
        for h in range(HPC):
            af[:, h * SEQP: h * SEQP + N] = afac_all[2 * c + h]
            bf[:, h * SEQP: h * SEQP + N] = bfac_all[2 * c + h]
        in_maps.append({
            "xT": xT,
            "wqT": np.ascontiguousarray(wq).astype(BF16NP),
            "wkT": np.ascontiguousarray(wk).astype(BF16NP),
            "wvT": np.ascontiguousarray(wv).astype(BF16NP),
            "qb": np.ascontiguousarray((qb_full[sl] * scale).reshape(128, 1)),
            "kb": np.ascontiguousarray(kb_full[sl].reshape(128, 1)),
            "pwT": np.ascontiguousarray(pw[:, sl].T).astype(BF16NP),
            "afac": np.ascontiguousarray(af).astype(BF16NP),
            "bfac": np.ascontiguousarray(bf).astype(BF16NP),
        })
    return in_maps


LAST_RESULTS = None


def kernel(x, qkv_weight, q_bias, k_bias, v_bias, proj_weight, proj_bias,
           rel_pos_table, res_h=512, res_w=512):
    global LAST_RESULTS
    if "nc" not in _CACHE:
        _CACHE["nc"] = build_nc()
    nc = _CACHE["nc"]

    in_maps = _prep_inputs(x, qkv_weight, q_bias, k_bias, v_bias, proj_weight,
                           rel_pos_table)
    trace = os.environ.get("KERNEL_TRACE", "0") == "1"
    res = run_bass_kernel_spmd(nc, in_maps, core_ids=list(range(NCORES)),
                               trace=trace)
    LAST_RESULTS = res

    total = np.zeros((C, BN), dtype=np.float32)
    for r in res.results:
        total += np.asarray(r["out_t"], dtype=np.float32)
    # v_bias is linear through attention + projection: fold on host.
    bias_eff = (np.asarray(proj_bias, dtype=np.float32)
                + np.asarray(proj_weight, dtype=np.float32)
                @ np.asarray(v_bias, dtype=np.float32))
    out = total.T + bias_eff
    return np.ascontiguousarray(out.reshape(B, N, C), dtype=np.float32)
